# revision 51
# baseline (speedup 1.0000x reference)
"""AutoCorrelation (Autoformer) kernel for 8 Trainium2 NeuronCores.

Full inputs: queries/keys/values (16, 2048, 512) fp32.
Returns (out, corr_out), both (16, 2048, 512) fp32, matching reference.

Sharding: data-parallel over batch B=16 -> 2 batches per core.

Kernel A per core: circular cross-correlation per channel via DFT-2048
factored as radix (128, 16): t = 16*t1 + t2, f = f1 + 128*f2.
fp16 pipeline with scale folding (forward weights carry 1/64 so the
spectral product P = Q*conj(K) fits fp16 range; inverse weights carry
64^2/L).

Conjugate symmetry: q,k real => P(-f) = conj(P(f)).  Only f1 in [1,64]
is carried through transpose / stage-S / pointwise / invS; the missing
conjugate half (f1 in [65,127]) is folded into the invB weights as a
factor c(f1)=2 on the real part (c=1 for the self-paired f1=64 column,
whose internal f2-pairs are stored redundantly).  The f1=0 "DC" family
(f = 128*f2) is computed on the otherwise-idle Pool engine via a
partition all-reduce over t1, a tiny DFT-16, pointwise, and inverse
DFT-16, then broadcast-accumulated into the invB PSUM with a
1-partition ones matmul.

Stages per (b, 256-channel d-half):
  stage-1 (contract t1): per-t2 twiddled DFT-128 matmuls, f1h out
  PE transpose (64-row blocks) -> T[(j,t2), (g, f1h)]
  stage-S (contract t2): block-diagonal I8 (x) DFT-16 matmuls
  pointwise P = Q * conj(K) on DVE (fp16 2x)
  invS fused with transpose -> Z[f1h, (go, pl, j, t2)] (one evac/psum)
  invB (contract f1h) with conj-folded +i twiddles; ACT evacs carry
  accum_out, which yields mean_value channel sums for free
PSUM->SBUF evacuations are split DVE/ACT by least-finish-time; output
DMAs ride ACT's queue, input loads ride SP's.

Kernel B (compiled per delta-set, shifts baked): out[b,l,:] =
  sum_k w[b,k] * v[b,(l+idx_k)%L,:] via merged shifted-identity matmuls
  accumulated in PSUM per 128-row output tile.
"""
import math
import numpy as np

L = 2048
N1 = 128   # t1
NF = 64    # carried f1 values: f1 = 1..64
N2 = 16    # t2 / f2
TOPK = int(1 * math.log(L))  # 7
NCORES = 8
SCALE = 64.0   # folded into forward weights; undone in inverse weights

_cache = {}


# ---------------------------------------------------------------- constants
def _consts():
    if "consts" in _cache:
        return _cache["consts"]
    t1 = np.arange(N1)
    f1 = np.arange(1, NF + 1)      # carried half-spectrum rows
    t2 = np.arange(N2)
    f2 = np.arange(N2)

    # stage-1 fwd: W1[t2][t1, f1h] = exp(-2i pi (t1 f1/128 + t2 f1/2048))/S
    th = (2 * np.pi) * (np.einsum("a,b->ab", t1, f1) / N1)[None, :, :] \
        + (2 * np.pi) * (np.einsum("a,b->ab", t2, f1) / L)[:, None, :]
    w1r = (np.cos(th) / SCALE).transpose(1, 0, 2).reshape(N1, N2 * NF)
    w1i = (-np.sin(th) / SCALE).transpose(1, 0, 2).reshape(N1, N2 * NF)

    # stage-S fwd blockdiag, j-major both sides:
    wr = np.cos(2 * np.pi * np.einsum("a,b->ab", t2, f2) / N2)
    wi = -np.sin(2 * np.pi * np.einsum("a,b->ab", t2, f2) / N2)
    bdr = np.kron(np.eye(8), wr)
    bdi = np.kron(np.eye(8), wi)
    bdmi = -bdi

    # invS blockdiag: BI[(j*16+f2), (j'*16+t2)] = delta exp(+2i pi f2 t2/16)
    vr = np.cos(2 * np.pi * np.einsum("a,b->ab", f2, t2) / N2)
    vi = np.sin(2 * np.pi * np.einsum("a,b->ab", f2, t2) / N2)
    bir_ = np.kron(np.eye(8), vr)
    bii = np.kron(np.eye(8), vi)
    bimi = -bii
    biri = np.concatenate([bir_, bii], axis=1)
    bimr = np.concatenate([bimi, bir_], axis=1)

    # invB with conjugate fold: corr += sum_f1h c(f1) Re[z e^{+i th}] S^2/L
    thi = (2 * np.pi / L) * np.einsum("a,bc->abc", f1,
                                      (16 * t1[None, :] + t2[:, None]))
    # thi[f1h, t2, t1]
    inv_scale = SCALE * SCALE / L    # = 2.0
    cfac = np.where(f1 == NF, 1.0, 2.0)[:, None, None]
    w1ir = (np.cos(thi) * inv_scale * cfac).reshape(NF, N2 * N1)
    w1iin = (-np.sin(thi) * inv_scale * cfac).reshape(NF, N2 * N1)

    iden = np.eye(NF)

    # DC family (f = 128*f2): forward DFT-16 over t2 (real/imag separate),
    # inverse DFT-16 back to t2 with S^2/L restore
    dc_r = (np.cos(2 * np.pi * np.einsum("a,b->ab", t2, f2) / N2) / SCALE)
    dc_i = (-np.sin(2 * np.pi * np.einsum("a,b->ab", t2, f2) / N2) / SCALE)
    idc_r = (np.cos(2 * np.pi * np.einsum("a,b->ab", f2, t2) / N2)
             * inv_scale)
    idc_mi = (-np.sin(2 * np.pi * np.einsum("a,b->ab", f2, t2) / N2)
              * inv_scale)
    ones1 = np.ones((1, 128))

    c = dict(w1r=w1r, w1i=w1i, bdr=bdr, bdi=bdi, bdmi=bdmi,
             biri=biri, bimr=bimr, w1ir=w1ir, w1iin=w1iin, iden=iden,
             dc_r=dc_r, dc_i=dc_i, idc_r=idc_r, idc_mi=idc_mi, ones1=ones1)
    c = {k: np.ascontiguousarray(v, np.float16) for k, v in c.items()}
    _cache["consts"] = c
    return c


CONST_SHAPES = {
    "w1r": (128, N2 * NF), "w1i": (128, N2 * NF),
    "w1ir": (NF, 2048), "w1iin": (NF, 2048),
    "bdr": (128, 128), "bdi": (128, 128), "bdmi": (128, 128),
    "biri": (128, 256), "bimr": (128, 256),
    "iden": (NF, NF),
    "dc_r": (16, 16), "dc_i": (16, 16),
    "idc_r": (16, 16), "idc_mi": (16, 16),
    "ones1": (1, 128),
}


def _make_evac(nc):
    """Greedy least-finish-time PSUM->SBUF evac across DVE/ACT."""
    load = {"v": 0.0, "a": 0.0}

    def cost(e, n, fp16):
        if e == "v":
            return (0.521 if fp16 else 1.042) * n + 125.0
        return 0.90 * n + 150.0

    def evac(dst, src, n, fp16=False):
        e = min(load, key=lambda x: load[x] + cost(x, n, fp16))
        load[e] += cost(e, n, fp16)
        if e == "v":
            nc.vector.tensor_copy(dst, src)
        else:
            nc.scalar.copy(dst, src)
        return e

    def preload(engine, ns):
        load[engine] += ns

    return evac, preload


# ---------------------------------------------------------------- kernel A
def _build_kernel_a():
    if "nc_a" in _cache:
        return _cache["nc_a"]
    import concourse.bacc as bacc
    import concourse.mybir as mybir
    from concourse import tile
    from concourse import tile_utils
    from concourse import bass_isa
    tile_utils.max_sbuf_usage = 206 * 1024

    f32 = mybir.dt.float32
    f16 = mybir.dt.float16
    nc = bacc.Bacc("TRN2", target_bir_lowering=False, debug=False,
                   num_devices=NCORES)
    q = nc.dram_tensor("q", [2, L, 512], f16, kind="ExternalInput").ap()
    k = nc.dram_tensor("k", [2, L, 512], f16, kind="ExternalInput").ap()
    cap = {n: nc.dram_tensor(n, list(s), f16, kind="ExternalInput").ap()
           for n, s in CONST_SHAPES.items()}
    corr = nc.dram_tensor("corr", [2, L, 512], f16, kind="ExternalOutput").ap()
    # per-(b, dh) channel-sums of corr: mean_value = sum over dh / 512
    mred = nc.dram_tensor("mred", [2, 2, 128, 16], f32,
                          kind="ExternalOutput").ap()

    W = 4096          # (t2, c) columns per (b, d-half) iteration
    WH = NOCT = 32    # octets (channel groups of 8)
    WT = NOCT * NF    # 2048: T/S/P tile free size (g, f1h)

    with tile.TileContext(nc) as tc:
        with tc.tile_pool(name="consts", bufs=1) as cpool, \
             tc.tile_pool(name="io", bufs=2) as io, \
             tc.tile_pool(name="work", bufs=2) as work, \
             tc.tile_pool(name="wz", bufs=1) as wzp, \
             tc.tile_pool(name="ps", bufs=8, space="PSUM") as psp:

            evac, preload = _make_evac(nc)
            mult = mybir.AluOpType.mult
            copyf = mybir.ActivationFunctionType.Copy
            cs = {}

            def load(st):
                b, dh = st["b"], st["dh"]
                csl = slice(dh * 256, (dh + 1) * 256)
                st["csl"] = csl
                # A[t1, (t2, c)], t = 16 t1 + t2; c = 256 channels
                a_q = io.tile([128, W], f16, tag="aq", name=f"aq{b}{dh}")
                a_k = io.tile([128, W], f16, tag="ak", name=f"ak{b}{dh}")
                # chunked over t2-quarters so stage1 starts on the first
                # quarter while the rest stream in
                qv = q[b, :, csl].rearrange("(a t) c -> a t c", t=N2)
                kv = k[b, :, csl].rearrange("(a t) c -> a t c", t=N2)
                aqv = a_q.rearrange("p (t c) -> p t c", t=N2)
                akv = a_k.rearrange("p (t c) -> p t c", t=N2)
                for c4 in range(4):
                    tsl = slice(c4 * 4, (c4 + 1) * 4)
                    nc.sync.dma_start(aqv[:, tsl], qv[:, tsl])
                    nc.sync.dma_start(akv[:, tsl], kv[:, tsl])
                st["a_q"], st["a_k"] = a_q, a_k

            def stage1(st):
                # contract t1 -> B[f1h, (cc, t2)] c-major, f1 = 1..64
                preload("v", 7000.0)    # pointwise lives on DVE
                preload("a", 6100.0)    # this iter's invB+mean (ACT)
                a_q, a_k = st["a_q"], st["a_k"]
                bq_r = wzp.tile([NF, W], f16, tag="b0", name="bq_r")
                bq_i = wzp.tile([NF, W], f16, tag="b1", name="bq_i")
                bk_r = wzp.tile([NF, W], f16, tag="b2", name="bk_r")
                bk_i = wzp.tile([NF, W], f16, tag="b3", name="bk_i")
                for tp in range(8):   # 2 t2 per psum bank
                    pss = [psp.tile([NF, 512], f32, tag="ps",
                                    name=f"ps_s1_{i}") for i in range(4)]
                    for ti in range(2):
                        t2v = tp * 2 + ti
                        wsl = slice(t2v * NF, (t2v + 1) * NF)
                        asl = slice(t2v * 256, (t2v + 1) * 256)
                        psl = slice(ti * 256, (ti + 1) * 256)
                        nc.tensor.matmul(pss[0][:, psl], cs["w1r"][:, wsl],
                                         a_q[:, asl], start=True, stop=True)
                        nc.tensor.matmul(pss[1][:, psl], cs["w1i"][:, wsl],
                                         a_q[:, asl], start=True, stop=True)
                        nc.tensor.matmul(pss[2][:, psl], cs["w1r"][:, wsl],
                                         a_k[:, asl], start=True, stop=True)
                        nc.tensor.matmul(pss[3][:, psl], cs["w1i"][:, wsl],
                                         a_k[:, asl], start=True, stop=True)
                    for pi, bp in enumerate((bq_r, bq_i, bk_r, bk_i)):
                        dst = bp.rearrange("p (cc t) -> p t cc", t=N2)
                        evac(dst[:, tp * 2:(tp + 1) * 2, :],
                             pss[pi].rearrange("p (ti cc) -> p ti cc",
                                               ti=2), 512)
                st.update(bq_r=bq_r, bq_i=bq_i, bk_r=bk_r, bk_i=bk_i)

            def dc_path(st):
                # f1=0 family on Pool: B0 = sum_t1 a; DFT-16; pointwise;
                # inverse DFT-16 -> corr_dc[(t2, c)] as a [1, 4096] stripe
                a_q, a_k = st["a_q"], st["a_k"]
                dcq = wzp.tile([128, W], f16, tag="dcq", name="dcq")
                dck = wzp.tile([128, W], f16, tag="dck", name="dck")
                nc.gpsimd.partition_all_reduce(dcq[:], a_q[:], 128,
                                               bass_isa.ReduceOp.add)
                nc.gpsimd.partition_all_reduce(dck[:], a_k[:], 128,
                                               bass_isa.ReduceOp.add)
                sq = wzp.tile([16, 256], f16, tag="dsq", name="dsq")
                sk = wzp.tile([16, 256], f16, tag="dsk", name="dsk")
                nc.sync.dma_start(
                    sq[:], dcq[0:1, :].rearrange("p (t c) -> p t c", t=N2))
                nc.sync.dma_start(
                    sk[:], dck[0:1, :].rearrange("p (t c) -> p t c", t=N2))
                zqr = wzp.tile([16, 256], f16, tag="zqr", name="zqr")
                zqi = wzp.tile([16, 256], f16, tag="zqi", name="zqi")
                zkr = wzp.tile([16, 256], f16, tag="zkr", name="zkr")
                zki = wzp.tile([16, 256], f16, tag="zki", name="zki")
                for (w_, src, dst) in ((cs["dc_r"], sq, zqr),
                                       (cs["dc_i"], sq, zqi),
                                       (cs["dc_r"], sk, zkr),
                                       (cs["dc_i"], sk, zki)):
                    ps = psp.tile([16, 256], f32, tag="ps", name="ps_dc")
                    nc.tensor.matmul(ps[:], w_[:], src[:],
                                     start=True, stop=True)
                    nc.vector.tensor_copy(dst[:], ps[:])
                pdr = wzp.tile([16, 256], f16, tag="pdr", name="pdr")
                pdi = wzp.tile([16, 256], f16, tag="pdi", name="pdi")
                tdc = wzp.tile([16, 256], f16, tag="tdc", name="tdc")
                nc.vector.tensor_tensor(pdr[:], zqr[:], zkr[:], mult)
                nc.vector.tensor_tensor(tdc[:], zqi[:], zki[:], mult)
                nc.vector.tensor_add(pdr[:], pdr[:], tdc[:])
                nc.vector.tensor_tensor(pdi[:], zqi[:], zkr[:], mult)
                nc.vector.tensor_tensor(tdc[:], zqr[:], zki[:], mult)
                nc.vector.tensor_sub(pdi[:], pdi[:], tdc[:])
                ps = psp.tile([16, 256], f32, tag="ps", name="ps_dci")
                nc.tensor.matmul(ps[:], cs["idc_r"][:], pdr[:],
                                 start=True, stop=False)
                nc.tensor.matmul(ps[:], cs["idc_mi"][:], pdi[:],
                                 start=False, stop=True)
                cdc = wzp.tile([16, 256], f16, tag="cdc", name="cdc")
                nc.vector.tensor_copy(cdc[:], ps[:])
                cdc1 = wzp.tile([1, W], f16, tag="cdc1", name="cdc1")
                nc.sync.dma_start(
                    cdc1.rearrange("p (t c) -> p t c", t=N2), cdc[:])
                st["cdc1"] = cdc1

            def transpose(st):
                # out T[(j*16+t2), (g, f1h)]; input blocks are [64, 128]
                t_q_r = work.tile([128, WT], f16, tag="t0", name="t_q_r")
                t_q_i = work.tile([128, WT], f16, tag="t1", name="t_q_i")
                t_k_r = work.tile([128, WT], f16, tag="t2", name="t_k_r")
                t_k_i = work.tile([128, WT], f16, tag="t3", name="t_k_i")
                for (bp, tt) in ((st["bq_r"], t_q_r), (st["bq_i"], t_q_i),
                                 (st["bk_r"], t_k_r), (st["bk_i"], t_k_i)):
                    for gg in range(4):
                        ps = psp.tile([128, 8 * NF], f16, tag="ps",
                                      name="ps_t")
                        for gi in range(8):
                            g = gg * 8 + gi
                            nc.tensor.transpose(
                                ps[:, gi * NF:(gi + 1) * NF],
                                bp[:, g * 128:(g + 1) * 128],
                                cs["iden"][:])
                        evac(tt[:, gg * 512:(gg + 1) * 512], ps[:], 512,
                             fp16=True)
                st.update(t_q_r=t_q_r, t_q_i=t_q_i, t_k_r=t_k_r, t_k_i=t_k_i)

            def stage_s(st):
                # contract t2 (blockdiag), 512-col chunks over (g, f1h)
                sq_r = work.tile([128, WT], f16, tag="s0", name="sq_r")
                sq_i = work.tile([128, WT], f16, tag="s1", name="sq_i")
                sk_r = work.tile([128, WT], f16, tag="s2", name="sk_r")
                sk_i = work.tile([128, WT], f16, tag="s3", name="sk_i")
                for (tr, ti_, sr, si) in (
                        (st["t_q_r"], st["t_q_i"], sq_r, sq_i),
                        (st["t_k_r"], st["t_k_i"], sk_r, sk_i)):
                    for ch in range(4):
                        sl = slice(ch * 512, (ch + 1) * 512)
                        psr = psp.tile([128, 512], f32, tag="ps", name="ps_sr")
                        psi = psp.tile([128, 512], f32, tag="ps", name="ps_si")
                        nc.tensor.matmul(psr[:], cs["bdr"][:], tr[:, sl],
                                         start=True, stop=False)
                        nc.tensor.matmul(psr[:], cs["bdmi"][:], ti_[:, sl],
                                         start=False, stop=True)
                        nc.tensor.matmul(psi[:], cs["bdi"][:], tr[:, sl],
                                         start=True, stop=False)
                        nc.tensor.matmul(psi[:], cs["bdr"][:], ti_[:, sl],
                                         start=False, stop=True)
                        evac(sr[:, sl], psr[:], 512)
                        evac(si[:, sl], psi[:], 512)
                st.update(sq_r=sq_r, sq_i=sq_i, sk_r=sk_r, sk_i=sk_i)

            def pointwise(st, g0=0, g1=NOCT):
                if "p_r" not in st:
                    st["p_r"] = work.tile([128, WT], f16, tag="t0", name="p_r")
                    st["p_i"] = work.tile([128, WT], f16, tag="t1", name="p_i")
                    st["tm"] = work.tile([128, WT], f16, tag="t2", name="tm")
                sl = slice(g0 * NF, g1 * NF)
                p_r, p_i, tm = st["p_r"][:, sl], st["p_i"][:, sl], \
                    st["tm"][:, sl]
                sq_r, sq_i = st["sq_r"][:, sl], st["sq_i"][:, sl]
                sk_r, sk_i = st["sk_r"][:, sl], st["sk_i"][:, sl]
                nc.vector.tensor_tensor(p_r, sq_r, sk_r, mult)
                nc.vector.tensor_tensor(tm, sq_i, sk_i, mult)
                nc.vector.tensor_add(p_r, p_r, tm)
                nc.vector.tensor_tensor(p_i, sq_i, sk_r, mult)
                nc.vector.tensor_tensor(tm, sq_r, sk_i, mult)
                nc.vector.tensor_sub(p_i, p_i, tm)

            def inv_s(st, g0=0, g1=NOCT):
                # fused invS + transpose: Z[f1h, (go, pl, j, t2)], one
                # 512-col evac per psum
                if "zz" not in st:
                    st["zz"] = wzp.tile([NF, 2 * W], f16, tag="gz", name="zz")
                zz = st["zz"]
                p_r, p_i = st["p_r"], st["p_i"]
                for g2 in range(g0 // 2, g1 // 2):
                    ps = psp.tile([NF, 512], f32, tag="ps", name="ps_is")
                    for gi in range(2):
                        g = g2 * 2 + gi
                        gsl = slice(g * NF, (g + 1) * NF)
                        osl = slice(gi * 256, (gi + 1) * 256)
                        nc.tensor.matmul(ps[:, osl], p_r[:, gsl],
                                         cs["biri"][:], start=True, stop=False)
                        nc.tensor.matmul(ps[:, osl], p_i[:, gsl],
                                         cs["bimr"][:], start=False, stop=True)
                    evac(zz[:, g2 * 512:(g2 + 1) * 512], ps[:], 512)

            def inv_b(st, tail=False):
                # invB: per t2 (contract f1h, conj-folded weights); the DC
                # stripe broadcast-accumulates via a 1-partition ones matmul;
                # ACT evacs carry accum_out for the mean channel-sums.  In
                # the drain (tail=True) the odd-t2 evacs go to the otherwise
                # idle DVE, with a tensor_reduce supplying their sums.
                b, dh = st["b"], st["dh"]
                zv = st["zz"].rearrange("p (go pl j t) -> p pl t go j",
                                        pl=2, j=8, t=N2)
                cdc1 = st["cdc1"]
                cdcv = cdc1.rearrange("p (t c) -> p t c", t=N2)
                c_sb = work.tile([128, W], f16, tag="c0", name="c_sb")
                red = work.tile([128, 16], f32, tag="r0", name="red")
                for tp in range(8):   # 2 t2 per bank
                    ps = psp.tile([128, 512], f32, tag="ps", name="ps_ib")
                    for ti in range(2):
                        t2v = tp * 2 + ti
                        wsl = slice(t2v * 128, (t2v + 1) * 128)
                        osl = slice(ti * 256, (ti + 1) * 256)
                        nc.tensor.matmul(ps[:, osl], cs["w1ir"][:, wsl],
                                         zv[:, 0, t2v], start=True, stop=False)
                        nc.tensor.matmul(ps[:, osl], cs["w1iin"][:, wsl],
                                         zv[:, 1, t2v], start=False, stop=False)
                        nc.tensor.matmul(ps[:, osl], cs["ones1"][:],
                                         cdcv[:, t2v], start=False, stop=True)
                    for ti in range(2):
                        t2v = tp * 2 + ti
                        osl = slice(ti * 256, (ti + 1) * 256)
                        nc.scalar.activation(
                            c_sb[:, t2v * 256:(t2v + 1) * 256], ps[:, osl],
                            copyf, accum_out=red[:, t2v:t2v + 1])
                    if tp in (3, 7):
                        # stream the finished t2-half out immediately
                        h = tp // 4
                        cv3 = c_sb.rearrange("p (t c) -> p t c", t=N2)
                        dst = corr[b, :, st["csl"]].rearrange(
                            "(a t) c -> a t c", t=N2)
                        nc.scalar.dma_start(dst[:, h * 8:(h + 1) * 8],
                                            cv3[:, h * 8:(h + 1) * 8])
                nc.scalar.dma_start(mred[b, dh], red[:])

            # software pipeline: back half of iter i interleaves into the
            # front half of iter i+1
            iters = [{"b": b, "dh": dh} for b in range(2) for dh in range(2)]

            def load_consts(names):
                for n in names:
                    t = cpool.tile(list(CONST_SHAPES[n]), f16, tag=n, name=n)
                    nc.sync.dma_start(t[:], cap[n][:])
                    cs[n] = t

            # stage1's weights first, then the first input tiles, then the
            # constants used by later stages
            load_consts(("w1r", "w1i", "iden"))
            load(iters[0])
            load_consts(("bdr", "bdi", "bdmi", "biri", "bimr",
                         "w1ir", "w1iin", "dc_r", "dc_i", "idc_r",
                         "idc_mi", "ones1"))
            prev = None
            for st in iters:
                if "a_q" not in st:
                    load(st)
                stage1(st)
                if prev is not None:
                    pointwise(prev)
                transpose(st)
                if prev is not None:
                    inv_s(prev)
                stage_s(st)
                if prev is not None:
                    inv_b(prev)
                dc_path(st)
                prev = st
            # drain the last iteration in two half-width waves so its
            # pointwise/invS/invB overlap each other
            pointwise(prev, 0, NOCT // 2)
            inv_s(prev, 0, NOCT // 2)
            pointwise(prev, NOCT // 2, NOCT)
            inv_s(prev, NOCT // 2, NOCT)
            inv_b(prev, tail=True)

    nc.compile()
    _cache["nc_a"] = nc
    return nc


# ---------------------------------------------------------------- kernel B
def _roll_deltas(idx):
    """Source-tile offsets used by the shifted-identity decomposition."""
    ds = set()
    for ix in idx:
        d, r = int(ix) >> 7, int(ix) & 127
        ds.add(d % 16)
        if r != 0:
            ds.add((d + 1) % 16)
    return sorted(ds)


def _roll_matrices(idx, w_b):
    """Per batch: merged shifted-identity matrices M_delta[src_p, dst_p]."""
    deltas = _roll_deltas(idx)
    dpos = {d: i for i, d in enumerate(deltas)}
    m = np.zeros((len(deltas), 128, 128), np.float32)
    for ki, ix in enumerate(idx):
        d, r = int(ix) >> 7, int(ix) & 127
        wv = float(w_b[ki])
        # piece 1: dst_p in [0, 128-r), src_p = dst_p + r, tile d
        for pd in range(128 - r):
            m[dpos[d % 16], pd + r, pd] += wv
        # piece 2: dst_p in [128-r, 128), src_p = dst_p + r - 128, tile d+1
        if r != 0:
            for pd in range(128 - r, 128):
                m[dpos[(d + 1) % 16], pd + r - 128, pd] += wv
    return m


def _load_order(deltas):
    """Order to DMA v tile-PAIRS (2 contiguous tiles per DMA) so output
    rows unblock earliest."""
    rank = {}
    nxt = 0
    for lt in range(16):
        for d in deltas:
            src = (lt + d) % 16
            if src not in rank:
                rank[src] = nxt
                nxt += 1
    for src in range(16):
        if src not in rank:
            rank[src] = nxt
            nxt += 1
    pairs = sorted(range(8), key=lambda pr: min(rank[2 * pr],
                                                rank[2 * pr + 1]))
    return pairs


def _build_kernel_b(idx):
    key = ("nc_b", tuple(_roll_deltas(idx)))
    if key in _cache:
        return _cache[key]
    import concourse.bacc as bacc
    import concourse.mybir as mybir
    from concourse import tile

    deltas = _roll_deltas(idx)
    nd = len(deltas)
    f32 = mybir.dt.float32
    f16 = mybir.dt.float16
    nc = bacc.Bacc("TRN2", target_bir_lowering=False, debug=False,
                   num_devices=NCORES)
    v = nc.dram_tensor("v", [2, L, 512], f16, kind="ExternalInput").ap()
    # host pre-transposes sm to [src_p, b, di, dst_p] for wide descriptors
    sm = nc.dram_tensor("sm", [128, 2, nd, 128], f16,
                        kind="ExternalInput").ap()
    outp = nc.dram_tensor("outp", [2, L, 512], f16, kind="ExternalOutput").ap()

    with tile.TileContext(nc) as tc:
        with tc.tile_pool(name="consts", bufs=1) as cpool, \
             tc.tile_pool(name="vp", bufs=2) as vp, \
             tc.tile_pool(name="work", bufs=4) as work, \
             tc.tile_pool(name="ps", bufs=8, space="PSUM") as psp:
            smt = cpool.tile([128, 2 * nd * 128], f16, tag="smt")
            nc.sync.dma_start(
                smt.rearrange("p (b di c) -> p b di c", b=2, di=nd), sm[:])

            for b in range(2):
                # one DMA per PAIR of contiguous v tiles: fewer HWDGE
                # round-trips in the lead-in, same unblocking granularity
                vt = [vp.tile([128, 2 * 512], f16, tag=f"v{j}",
                              name=f"vt{j}") for j in range(8)]
                for pr in _load_order(deltas):
                    nc.sync.dma_start(
                        vt[pr].rearrange("p (lt d) -> p lt d", lt=2),
                        v[b, pr * 256:(pr + 1) * 256, :]
                        .rearrange("(lt p) d -> p lt d", p=128))
                for ltg in range(4):
                    # st covers 4 output tiles -> one batched out-DMA
                    st = work.tile([128, 4 * 512], f16, tag="st",
                                   name=f"st{b}{ltg}")
                    for lti in range(4):
                        lt = ltg * 4 + lti
                        ps = psp.tile([128, 512], f32, tag="ps", name="ps_b")
                        for di in range(nd):
                            wslc = slice((b * nd + di) * 128,
                                         (b * nd + di) * 128 + 128)
                            src = (lt + deltas[di]) % 16
                            vsl = vt[src // 2][:, (src % 2) * 512:
                                               (src % 2) * 512 + 512]
                            nc.tensor.matmul(ps[:], smt[:, wslc], vsl,
                                             start=(di == 0),
                                             stop=(di == nd - 1))
                        dst = st[:, lti * 512:(lti + 1) * 512]
                        if lt % 2 == 0:
                            nc.vector.tensor_copy(dst, ps[:])
                        else:
                            nc.scalar.copy(dst, ps[:])
                    nc.scalar.dma_start(
                        outp[b].rearrange("(lt p) d -> p lt d", p=128)
                        [:, ltg * 4:(ltg + 1) * 4],
                        st.rearrange("p (lt d) -> p lt d", lt=4))
    nc.compile()
    _cache[key] = nc
    return nc


# ---------------------------------------------------------------- host glue
def _mean_value_from_mred(mreds):
    """mreds: list of 8 arrays (2, 2, 128, 16) -> mean_value (16, 2048)."""
    mv = np.zeros((16, L), np.float64)
    for bp in range(NCORES):
        m = mreds[bp].astype(np.float64)        # (b, dh, a, t2)
        s = m.sum(axis=1)                       # (b, a, t2); l = 16a + t2
        for b in range(2):
            mv[bp * 2 + b] = s[b].reshape(L) / 512.0
    return mv.astype(np.float32)


def _softmax(x):
    m = x.max(axis=-1, keepdims=True)
    e = np.exp(x - m)
    return e / e.sum(axis=-1, keepdims=True)


def kernel(queries, keys, values):
    from concourse.bass_utils import run_bass_kernel_spmd

    queries = np.ascontiguousarray(queries, np.float16)
    keys = np.ascontiguousarray(keys, np.float16)
    values = np.ascontiguousarray(values, np.float16)

    cs = _consts()
    nc_a = _build_kernel_a()
    in_maps = []
    for bp in range(NCORES):
        m = {"q": queries[bp * 2:bp * 2 + 2], "k": keys[bp * 2:bp * 2 + 2]}
        m.update(cs)
        in_maps.append(m)
    res_a = run_bass_kernel_spmd(nc_a, in_maps, list(range(NCORES)))

    corr_out = np.empty((16, L, 512), np.float32)
    for bp in range(NCORES):
        corr_out[bp * 2:bp * 2 + 2] = res_a.results[bp]["corr"]

    mv = _mean_value_from_mred([res_a.results[bp]["mred"]
                                for bp in range(NCORES)])
    gmean = mv.mean(axis=0)
    idx = np.argsort(-gmean, kind="stable")[:TOPK]
    weights = mv[:, idx]                       # (16, k)
    tmp_corr = _softmax(weights)               # (16, k)

    # kernel B
    nc_b = _build_kernel_b(idx)
    in_maps_b = []
    for bp in range(NCORES):
        smv = np.stack([_roll_matrices(idx, tmp_corr[bp * 2 + b])
                        for b in range(2)])           # (2, nd, src_p, dst_p)
        smv = np.ascontiguousarray(smv.transpose(2, 0, 1, 3),
                                   np.float16)        # (src_p, b, di, dst_p)
        in_maps_b.append({"v": values[bp * 2:bp * 2 + 2], "sm": smv})
    res_b = run_bass_kernel_spmd(nc_b, in_maps_b, list(range(NCORES)))

    out = np.empty((16, L, 512), np.float32)
    for bp in range(NCORES):
        out[bp * 2:bp * 2 + 2] = res_b.results[bp]["outp"]

    return out, corr_out


def timed_run(inputs):
    """No NTFF profiling hook exists under this axon client, so report the
    cost-model (TimelineSim) per-core execution time for both kernels."""
    import numpy as np
    from concourse.timeline_sim import TimelineSim
    queries = np.ascontiguousarray(inputs["queries"], np.float16)
    keys = np.ascontiguousarray(inputs["keys"], np.float16)
    from concourse.bass_utils import run_bass_kernel_spmd
    cs = _consts()
    nc_a = _build_kernel_a()
    in_maps = []
    for bp in range(NCORES):
        m = {"q": queries[bp * 2:bp * 2 + 2], "k": keys[bp * 2:bp * 2 + 2]}
        m.update(cs)
        in_maps.append(m)
    res_a = run_bass_kernel_spmd(nc_a, in_maps, list(range(NCORES)))
    mv = _mean_value_from_mred([res_a.results[bp]["mred"]
                                for bp in range(NCORES)])
    gmean = mv.mean(axis=0)
    idx = np.argsort(-gmean, kind="stable")[:TOPK]
    nc_b = _build_kernel_b(idx)
    ta = TimelineSim(nc_a).simulate()
    tb = TimelineSim(nc_b).simulate()
    print(f"  kernel A (cost model): {ta} ns")
    print(f"  kernel B (cost model): {tb} ns")
    return ta + tb
